# revision 8
# baseline (speedup 1.0000x reference)
import sys
import numpy as np

sys.path.insert(0, "/opt/trn_rl_repo")

B, T, IN, H = 256, 512, 256, 1024
NCORES = 8
BL = B // NCORES  # 32 batch rows per core
KH = H // 128  # 8
KI = IN // 128  # 2

_CACHE = {}


def _build(repeat=1):
    import concourse.bass as bass
    import concourse.tile as tile
    from concourse import bacc, mybir
    from concourse.bass import ds, ts

    nc = bacc.Bacc(
        "TRN2",
        target_bir_lowering=False,
        debug=False,
        enable_asserts=False,
        num_devices=NCORES,
    )
    f32 = mybir.dt.float32

    y0T_d = nc.dram_tensor("y0T", [IN, BL], f32, kind="ExternalInput").ap()
    wih0_d = nc.dram_tensor("w_ih0t", [IN, H], f32, kind="ExternalInput").ap()
    whh0_d = nc.dram_tensor("w_hh0t", [H, H], f32, kind="ExternalInput").ap()
    wih1_d = nc.dram_tensor("w_ih1t", [H, H], f32, kind="ExternalInput").ap()
    whh1_d = nc.dram_tensor("w_hh1t", [H, H], f32, kind="ExternalInput").ap()
    fcw_d = nc.dram_tensor("fc_wt", [H, IN], f32, kind="ExternalInput").ap()
    b0_d = nc.dram_tensor("bias0", [H, 1], f32, kind="ExternalInput").ap()
    b1_d = nc.dram_tensor("bias1", [H, 1], f32, kind="ExternalInput").ap()
    fcb_d = nc.dram_tensor("fc_bias", [IN, 1], f32, kind="ExternalInput").ap()
    # y_out_a[c][j] = y_{2j+1} feature chunk c; y_out_b[c][j] = y_{2j+2}
    ya_d = [
        nc.dram_tensor(f"y_out_a{c}", [T // 2, 128, BL], f32, kind="ExternalOutput").ap()
        for c in range(KI)
    ]
    yb_d = [
        nc.dram_tensor(f"y_out_b{c}", [T // 2, 128, BL], f32, kind="ExternalOutput").ap()
        for c in range(KI)
    ]

    Tanh = mybir.ActivationFunctionType.Tanh
    Ident = mybir.ActivationFunctionType.Identity

    with tile.TileContext(nc) as tc:
        with (
            tc.tile_pool(name="weights", bufs=1) as wpool,
            tc.tile_pool(name="state", bufs=1) as spool,
            tc.tile_pool(name="psum", bufs=1, space="PSUM") as ppool,
        ):
            wih0 = [wpool.tile([128, H], f32, name=f"wih0_{k}") for k in range(KI)]
            whh0 = [wpool.tile([128, H], f32, name=f"whh0_{k}") for k in range(KH)]
            wih1 = [wpool.tile([128, H], f32, name=f"wih1_{k}") for k in range(KH)]
            whh1 = [wpool.tile([128, H], f32, name=f"whh1_{k}") for k in range(KH)]
            fcw = [wpool.tile([128, IN], f32, name=f"fcw_{k}") for k in range(KH)]
            b0 = [wpool.tile([128, 1], f32, name=f"b0_{k}") for k in range(KH)]
            b1 = [wpool.tile([128, 1], f32, name=f"b1_{k}") for k in range(KH)]
            fcb = [wpool.tile([128, 1], f32, name=f"fcb_{k}") for k in range(KI)]

            for k in range(KI):
                nc.sync.dma_start(wih0[k], wih0_d[k * 128 : (k + 1) * 128, :])
                nc.sync.dma_start(fcb[k], fcb_d[k * 128 : (k + 1) * 128, :])
            for k in range(KH):
                nc.sync.dma_start(whh0[k], whh0_d[k * 128 : (k + 1) * 128, :])
                nc.sync.dma_start(wih1[k], wih1_d[k * 128 : (k + 1) * 128, :])
                nc.sync.dma_start(whh1[k], whh1_d[k * 128 : (k + 1) * 128, :])
                nc.sync.dma_start(fcw[k], fcw_d[k * 128 : (k + 1) * 128, :])
                nc.sync.dma_start(b0[k], b0_d[k * 128 : (k + 1) * 128, :])
                nc.sync.dma_start(b1[k], b1_d[k * 128 : (k + 1) * 128, :])

            yA = [spool.tile([128, BL], f32, name=f"yA_{k}") for k in range(KI)]
            yB = [spool.tile([128, BL], f32, name=f"yB_{k}") for k in range(KI)]
            h0A = [spool.tile([128, BL], f32, name=f"h0A_{k}") for k in range(KH)]
            h0B = [spool.tile([128, BL], f32, name=f"h0B_{k}") for k in range(KH)]
            h1A = [spool.tile([128, BL], f32, name=f"h1A_{k}") for k in range(KH)]
            h1B = [spool.tile([128, BL], f32, name=f"h1B_{k}") for k in range(KH)]

            for k in range(KI):
                nc.sync.dma_start(yA[k], y0T_d[k * 128 : (k + 1) * 128, :])
            for m in range(KH):
                nc.any.memzero(h0A[m])
                nc.any.memzero(h1A[m])

            # one accumulation group per PSUM bank per half-step, and every
            # bank's group is stopped before any ACT reads it (start
            # pending-zeroes the whole 2KB region). ph1 is split over 4 banks
            # (2 chunks each) so tanh1/fc can start before all of L1 is done.
            ph0_all = ppool.tile([128, 16, BL], f32, name="ph0_all")
            ph1_ab = [
                ppool.tile([128, 16, BL], f32, name=f"ph1_b{b}") for b in range(4)
            ]
            py_all = ppool.tile([128, 16, BL], f32, name="py_all")
            ph0 = [ph0_all[:, m] for m in range(KH)]
            ph1 = [ph1_ab[m // 2][:, m % 2] for m in range(KH)]
            py = [py_all[:, m] for m in range(KI)]

            def half_step(sy, sh0, sh1, dy, dh0, dh1, out_d, j):
                # layer 0: whole-bank group; whh0 first (no new deps), wih0
                # last (needs sy from previous half-step's fc tail)
                for m in range(KH):
                    for k in range(KH):
                        nc.tensor.matmul(
                            ph0[m], whh0[k][:, ts(m, 128)], sh0[k],
                            start=(m == 0 and k == 0), stop=False,
                        )
                for m in range(KH):
                    for k in range(KI):
                        nc.tensor.matmul(
                            ph0[m], wih0[k][:, ts(m, 128)], sy[k],
                            start=False, stop=(m == KH - 1 and k == KI - 1),
                        )
                for m in range(KH):
                    nc.scalar.activation(dh0[m], ph0[m], Tanh, bias=b0[m])
                # layer 1 recurrent part first (only needs prev-step h1);
                # k-outer: each ph1 bank's group starts at its k=0 first touch
                for k in range(KH):
                    for m in range(KH):
                        nc.tensor.matmul(
                            ph1[m], whh1[k][:, ts(m, 128)], sh1[k],
                            start=(k == 0 and m % 2 == 0), stop=False,
                        )
                # layer 1 input part, m-outer: bank b (chunks 2b, 2b+1) stops
                # at chunk 2b+1's last k, then its tanh1 fires immediately
                for m in range(KH):
                    for k in range(KH):
                        nc.tensor.matmul(
                            ph1[m], wih1[k][:, ts(m, 128)], dh0[k],
                            start=False, stop=(m % 2 == 1 and k == KH - 1),
                        )
                    if m % 2 == 1:
                        nc.scalar.activation(dh1[m - 1], ph1[m - 1], Tanh, bias=b1[m - 1])
                        nc.scalar.activation(dh1[m], ph1[m], Tanh, bias=b1[m])
                # fc, k-outer consumes dh1 progressively
                for k in range(KH):
                    for c in range(KI):
                        nc.tensor.matmul(
                            py[c], fcw[k][:, ts(c, 128)], dh1[k],
                            start=(k == 0 and c == 0), stop=(k == KH - 1 and c == KI - 1),
                        )
                for c in range(KI):
                    nc.scalar.activation(dy[c], py[c], Ident, bias=fcb[c])
                    nc.sync.dma_start(out_d[c][ds(j, 1)], dy[c])

            with tc.For_i(0, repeat, 1):
                with tc.For_i(0, T // 2, 1) as j:
                    half_step(yA, h0A, h1A, yB, h0B, h1B, ya_d, j)
                    half_step(yB, h0B, h1B, yA, h0A, h1A, yb_d, j)

    nc.compile()
    return nc


def kernel(**inputs):
    from concourse import bass_utils

    if "nc" not in _CACHE:
        _CACHE["nc"] = _build()
    nc = _CACHE["nc"]

    y0 = np.asarray(inputs["y0"], dtype=np.float32)
    cat = np.ascontiguousarray
    common = {
        "w_ih0t": cat(np.asarray(inputs["W_ih0"], np.float32).T),
        "w_hh0t": cat(np.asarray(inputs["W_hh0"], np.float32).T),
        "w_ih1t": cat(np.asarray(inputs["W_ih1"], np.float32).T),
        "w_hh1t": cat(np.asarray(inputs["W_hh1"], np.float32).T),
        "fc_wt": cat(np.asarray(inputs["fc_W"], np.float32).T),
        "bias0": cat(
            (np.asarray(inputs["b_ih0"], np.float32) + np.asarray(inputs["b_hh0"], np.float32)).reshape(H, 1)
        ),
        "bias1": cat(
            (np.asarray(inputs["b_ih1"], np.float32) + np.asarray(inputs["b_hh1"], np.float32)).reshape(H, 1)
        ),
        "fc_bias": cat(np.asarray(inputs["fc_b"], np.float32).reshape(IN, 1)),
    }
    in_maps = []
    for c in range(NCORES):
        m = dict(common)
        m["y0T"] = cat(y0[c * BL : (c + 1) * BL, :].T)
        in_maps.append(m)

    res = bass_utils.run_bass_kernel_spmd(nc, in_maps, core_ids=list(range(NCORES)))
    _CACHE["last_result"] = res

    out = np.empty((B, T, IN), dtype=np.float32)
    for c in range(NCORES):
        r = res.results[c]
        a = np.concatenate([r["y_out_a0"], r["y_out_a1"]], axis=1)  # [256, 256, 32]
        bb = np.concatenate([r["y_out_b0"], r["y_out_b1"]], axis=1)
        ys = np.empty((T, IN, BL), dtype=np.float32)
        ys[0] = y0[c * BL : (c + 1) * BL, :].T
        ys[1::2] = a
        ys[2::2] = bb[: T // 2 - 1]
        out[c * BL : (c + 1) * BL] = ys.transpose(2, 0, 1)
    return out
